# revision 1
# baseline (speedup 1.0000x reference)
"""Causal self-attention (GQA + RoPE) on 8 Trainium2 NeuronCores.

Sharding: core c = (b, g) with b = c // 4 (batch), g = c % 4 (group of 4
consecutive Q heads; KV head g // 2). Each core computes the attention
output for its 4 heads and a partial out-projection through the matching
256-column slice of Wo. Host sums the 4 partials per batch and adds bo.

Per-core kernel, ch-major fused pipeline (all matmuls bf16):
  for ch in 0..3:  (512-wide t-chunk)
    kv proj ch -> gq bias evac (GpSimd) -> v transpose, k RoPE
    q proj ch (2 pair-slabs) + q RoPE
    attention qb=ch for 4 heads (scores pre-transposed sT[k,q]; exp on
      Scalar over paired 1024-wide PSUM tiles; AV with ones-column
      denominator; causal diag handled with GpSimd mask adds)
    out-projection for qb=ch
  x/weights stream per-chunk so the first matmul starts ~1us in and
  phase-1 work overlaps the scalar-bound attention of the prior chunk.
"""

import sys

for _p in ("/opt/trn_rl_repo", "/opt/pypackages"):
    if _p not in sys.path:
        sys.path.append(_p)

from contextlib import ExitStack

import numpy as np

import concourse.bacc as bacc
import concourse.mybir as mybir
import concourse.tile as tile
from concourse.bass import ts
from concourse.bass_utils import run_bass_kernel_spmd

B, T, C = 2, 2048, 1024
HQ, HKV, HD = 16, 2, 64
F32 = mybir.dt.float32
F32R = mybir.dt.float32r
FP8 = mybir.dt.float8e4
DR = mybir.MatmulPerfMode.DoubleRow
BF16 = mybir.dt.bfloat16
AF = mybir.ActivationFunctionType
NCC = C // 128  # 8 chunks of the contraction dim
NEG = -1.0e30
SCALE = 1.0 / 64.0  # the reference's double 1/sqrt(64) scaling


def _emit(nc, tc, ctx, d):
    sing = ctx.enter_context(tc.tile_pool(name="sing", bufs=1))

    xT_sb = [sing.tile([128, NCC, 512], BF16, name=f"x{c}") for c in range(4)]
    x0_sb = [sing.tile([128, 512], BF16, name=f"x0_{cc}") for cc in range(NCC)]

    def xch(ch, cc):
        return x0_sb[cc][:] if ch == 0 else xT_sb[ch][:, cc, :]
    wqkv_sb = sing.tile([128, NCC, 384], BF16)
    wo_sb = sing.tile([128, 2, C], BF16)
    bq_sb = sing.tile([128, 2], F32)
    bkv_sb = sing.tile([128, 1], F32)
    cs_sb = [sing.tile([128, 2, 512], BF16, name=f"cs{c}") for c in range(4)]
    cos_sb = [t[:, 0, :] for t in cs_sb]
    sin_sb = [t[:, 1, :] for t in cs_sb]
    r2t_sb = sing.tile([128, 128], BF16)
    id_sb = sing.tile([64, 64], BF16)
    dm_sb = sing.tile([128, 128], F32)
    qT_sb = sing.tile([128, 2, T], BF16)   # pair j: head 2j at parts 0:64, 2j+1 at 64:128
    kvT_sb = sing.tile([128, T], BF16)     # v at parts 0:64, k (pre-rope) at 64:128
    kz0_sb = sing.tile([128, T], BF16)     # roped k at 0:64, zeros at 64:128
    kz1_sb = sing.tile([128, T], BF16)     # zeros at 0:64, roped k at 64:128
    vA_sb = sing.tile([128, 16, 128], BF16)  # ones col 0, zeros, v at 64:128
    vA8_sb = sing.tile([128, 16, 128], FP8)  # fp8 copy for DoubleRow AV
    yT_sb = sing.tile([128, 2, T], BF16)   # normalized attention out, pair layout

    # input DMAs, ordered so the first projection can start ~1us in
    xp = d["xT"].ap()   # [128, 4*NCC*512] pre-swizzled (ch, cc) chunks
    csp = d["cs"].ap()  # [128, 4*2*512] pre-swizzled per-ch cos|sin
    nc.sync.dma_start(out=x0_sb[0][:], in_=xp[:, ts(0, 512)])
    nc.sync.dma_start(out=wqkv_sb[:], in_=d["wqkv"].ap())
    nc.sync.dma_start(out=bkv_sb[:], in_=d["bkv"].ap())
    nc.sync.dma_start(out=bq_sb[:], in_=d["bq"].ap())
    for cc in range(1, NCC):
        nc.sync.dma_start(out=x0_sb[cc][:], in_=xp[:, ts(cc, 512)])
    nc.sync.dma_start(out=id_sb[:], in_=d["ident"].ap())
    nc.sync.dma_start(out=cs_sb[0][:], in_=csp[:, ts(0, 1024)])
    nc.sync.dma_start(out=r2t_sb[:], in_=d["r2t"].ap())
    nc.sync.dma_start(out=dm_sb[:], in_=d["dmask"].ap())
    nc.sync.dma_start(out=xT_sb[1][:], in_=xp[:, ts(1, NCC * 512)])
    nc.sync.dma_start(out=cs_sb[1][:], in_=csp[:, ts(1, 1024)])
    nc.sync.dma_start(out=wo_sb[:], in_=d["wo"].ap())
    for ch in range(2, 4):
        nc.sync.dma_start(out=xT_sb[ch][:], in_=xp[:, ts(ch, NCC * 512)])
        nc.sync.dma_start(out=cs_sb[ch][:], in_=csp[:, ts(ch, 1024)])

    # preload the exp act table while the input DMAs stream
    warm = sing.tile([1, 1], F32)
    nc.scalar.activation(out=warm[:], in_=dm_sb[0:1, 0:1], func=AF.Exp, scale=1.0)

    # one-off constants (GpSimd so DVE stays free)
    nc.gpsimd.memset(vA_sb[:, :, 0:1], 1.0)
    nc.gpsimd.memset(vA_sb[:, :, 1:64], 0.0)
    nc.vector.tensor_copy(vA8_sb[:, :, 0:64], vA_sb[:, :, 0:64])
    nc.gpsimd.memset(kz0_sb[64:128, :], 0.0)
    nc.gpsimd.memset(kz1_sb[0:64, :], 0.0)

    pw = ctx.enter_context(tc.tile_pool(name="pw", bufs=2, space="PSUM"))
    psp = ctx.enter_context(tc.tile_pool(name="psp", bufs=2, space="PSUM"))
    pyp = ctx.enter_context(tc.tile_pool(name="pyp", bufs=2, space="PSUM"))
    ep = ctx.enter_context(tc.tile_pool(name="ep", bufs=6))
    tmp = ctx.enter_context(tc.tile_pool(name="tmp", bufs=2))
    nrm = ctx.enter_context(tc.tile_pool(name="nrm", bufs=2))
    ost = ctx.enter_context(tc.tile_pool(name="ost", bufs=3))

    for ch in range(4):
        tc_cols = ts(ch, 512)
        # ---- projections for this t-chunk (interleaved so the PE never
        # waits on a DVE evacuation) ----
        kvp = pw.tile([128, 512], F32, tag="w")
        for cc in range(NCC):
            nc.tensor.matmul(
                kvp[:], wqkv_sb[:, cc, 0:128], xch(ch, cc),
                start=(cc == 0), stop=(cc == NCC - 1),
            )
        nc.vector.tensor_scalar_add(kvT_sb[:, tc_cols], kvp[:], bkv_sb[:, 0:1])
        # q proj j=0 keeps the PE busy while the kv evac runs
        qp0 = pw.tile([128, 512], F32, tag="w")
        for cc in range(NCC):
            nc.tensor.matmul(
                qp0[:], wqkv_sb[:, cc, 128:256], xch(ch, cc),
                start=(cc == 0), stop=(cc == NCC - 1),
            )
        nc.vector.tensor_scalar_add(qT_sb[:, 0, tc_cols], qp0[:], bq_sb[:, 0:1])
        # k RoPE rotation + v transposes (kvT ready by now)
        kr = pw.tile([128, 512], F32, tag="w")
        nc.tensor.matmul(
            kr[0:64, :], r2t_sb[64:128, 64:128], kvT_sb[64:128, tc_cols],
            start=True, stop=True,
        )
        for r in range(4):
            c16 = 4 * ch + r
            pv = pw.tile([128, 64], BF16, tag="w")
            nc.tensor.transpose(pv[:], kvT_sb[0:64, ts(c16, 128)], id_sb[:])
            nc.vector.tensor_copy(vA_sb[:, c16, 64:128], pv[:])
            nc.vector.tensor_copy(vA8_sb[:, c16, 64:128], pv[:])
        # q RoPE rotation j=0 (qT j0 evac done during kr/pv)
        qr0 = pw.tile([128, 512], F32, tag="w")
        nc.tensor.matmul(
            qr0[:], r2t_sb[:], qT_sb[:, 0, tc_cols], start=True, stop=True,
        )
        # k rope vector math
        t1 = tmp.tile([64, 512], BF16, tag="t1")
        t2 = tmp.tile([64, 512], BF16, tag="t2")
        nc.vector.tensor_mul(t1[:], kvT_sb[64:128, tc_cols], cos_sb[ch][64:128, :])
        nc.vector.tensor_mul(t2[:], kr[0:64, :], sin_sb[ch][0:64, :])
        nc.vector.tensor_add(kz0_sb[0:64, tc_cols], t1[:], t2[:])
        nc.sync.dma_start(out=kz1_sb[64:128, tc_cols], in_=kz0_sb[0:64, tc_cols])
        # q proj j=1 on the PE while DVE does k-rope
        qp1 = pw.tile([128, 512], F32, tag="w")
        for cc in range(NCC):
            nc.tensor.matmul(
                qp1[:], wqkv_sb[:, cc, 256:384], xch(ch, cc),
                start=(cc == 0), stop=(cc == NCC - 1),
            )
        nc.vector.tensor_scalar_add(qT_sb[:, 1, tc_cols], qp1[:], bq_sb[:, 1:2])
        # q rope vector math j=0
        q1 = tmp.tile([128, 512], BF16, tag="q1")
        q2 = tmp.tile([128, 512], BF16, tag="q2")
        nc.vector.tensor_mul(q1[:], qT_sb[:, 0, tc_cols], cos_sb[ch][:])
        nc.vector.tensor_mul(q2[:], qr0[:], sin_sb[ch][:])
        nc.vector.tensor_add(qT_sb[:, 0, tc_cols], q1[:], q2[:])
        qr1 = pw.tile([128, 512], F32, tag="w")
        nc.tensor.matmul(
            qr1[:], r2t_sb[:], qT_sb[:, 1, tc_cols], start=True, stop=True,
        )
        q3 = tmp.tile([128, 512], BF16, tag="q1")
        q4 = tmp.tile([128, 512], BF16, tag="q2")
        nc.vector.tensor_mul(q3[:], qT_sb[:, 1, tc_cols], cos_sb[ch][:])
        nc.vector.tensor_mul(q4[:], qr1[:], sin_sb[ch][:])
        nc.vector.tensor_add(qT_sb[:, 1, tc_cols], q3[:], q4[:])

        # ---- attention for qb = ch ----
        qb = ch
        qcols = ts(qb, 512)
        for h in range(4):
            j, b0 = h // 2, (h % 2) * 64
            kz = kz0_sb if h % 2 == 0 else kz1_sb
            py = pyp.tile([128, 512], F32, tag="y")
            # full 128x512 blocks below the diagonal band, in pairs
            for p in range(2 * qb):
                st = psp.tile([128, 2, 512], F32, tag="s")
                for i in range(2):
                    nc.tensor.matmul(
                        st[:, i, :], kz[:, ts(2 * p + i, 128)],
                        qT_sb[:, j, qcols], start=True, stop=True,
                    )
                e8 = ep.tile([128, 2, 512], FP8, tag="e8")
                nc.scalar.activation(out=e8[:], in_=st[:], func=AF.Exp, scale=SCALE)
                nc.tensor.matmul(
                    py[:], vA8_sb[:, 2 * p:2 * p + 2, :], e8[:],
                    start=(p == 0), stop=False, perf_mode=DR,
                )
            # diagonal band: chunk 4qb+r covers q in [kb*128, (qb+1)*512);
            # only its first 128 columns straddle the diagonal -> masked
            for ra in (0, 2):
                rb = ra + 1
                wa, wb = 512 - 128 * ra, 512 - 128 * rb
                st = psp.tile([128, 1024], F32, tag="s")
                nc.tensor.matmul(
                    st[:, 0:wa], kz[:, ts(4 * qb + ra, 128)],
                    qT_sb[:, j, qb * 512 + 128 * ra: (qb + 1) * 512],
                    start=True, stop=True,
                )
                nc.tensor.matmul(
                    st[:, wa:wa + wb], kz[:, ts(4 * qb + rb, 128)],
                    qT_sb[:, j, qb * 512 + 128 * rb: (qb + 1) * 512],
                    start=True, stop=True,
                )
                nc.vector.tensor_add(st[:, 0:128], st[:, 0:128], dm_sb[:])
                nc.vector.tensor_add(st[:, wa:wa + 128], st[:, wa:wa + 128], dm_sb[:])
                e = ep.tile([128, 1024], BF16, tag="e")
                nc.scalar.activation(
                    out=e[:, 0:wa + wb], in_=st[:, 0:wa + wb], func=AF.Exp, scale=SCALE,
                )
                nc.tensor.matmul(
                    py[:, 128 * ra:512], vA_sb[:, 4 * qb + ra, :], e[:, 0:wa],
                    start=(qb == 0 and ra == 0), stop=False,
                )
                nc.tensor.matmul(
                    py[:, 128 * rb:512], vA_sb[:, 4 * qb + rb, :], e[:, wa:wa + wb],
                    start=False, stop=(rb == 3),
                )
            # normalize: 1/den (DVE), broadcast across partitions (GpSimd),
            # then scale while evacuating the AV PSUM (DVE)
            rdr = nrm.tile([1, 512], F32, tag="rdr")
            nc.vector.reciprocal_approx_fast(rdr[:], py[0:1, :])
            pbs = nrm.tile([64, 512], F32, tag="pbs")
            nc.gpsimd.partition_broadcast(pbs[:], rdr[:])
            nc.vector.tensor_mul(
                yT_sb[b0:b0 + 64, j, qcols], py[64:128, :], pbs[:],
            )
        # ---- out projection for this q-block ----
        for tq in range(4 * qb, 4 * qb + 4):
            for cf in range(2):
                po = pw.tile([128, 512], F32, tag="w")
                for j in range(2):
                    nc.tensor.matmul(
                        po[:], yT_sb[:, j, ts(tq, 128)], wo_sb[:, j, ts(cf, 512)],
                        start=(j == 0), stop=(j == 1),
                    )
                ob = ost.tile([128, 512], BF16, tag="ob")
                nc.vector.tensor_copy(ob[:], po[:])
                nc.sync.dma_start(out=d["out"].ap()[ts(tq, 128), ts(cf, 512)], in_=ob[:])


def build_program():
    nc = bacc.Bacc("TRN2", target_bir_lowering=False, debug=False, num_devices=8)
    d = {}
    BF_IN = {"xT", "cs", "wqkv", "wo", "r2t", "ident"}
    for name, shape in [
        ("xT", [128, 4 * NCC * 512]), ("cs", [128, 4 * 2 * 512]), ("wqkv", [128, NCC * 384]),
        ("bq", [128, 2]), ("bkv", [128, 1]), ("wo", [128, 2 * C]),
        ("r2t", [128, 128]),
        ("ident", [64, 64]), ("dmask", [128, 128]),
    ]:
        dt = BF16 if name in BF_IN else F32
        d[name] = nc.dram_tensor(name, shape, dt, kind="ExternalInput")
    d["out"] = nc.dram_tensor("out", [T, C], BF16, kind="ExternalOutput")
    with tile.TileContext(nc) as tc, ExitStack() as ctx:
        _emit(nc, tc, ctx, d)
    nc.compile()
    return nc


def host_prep(inputs):
    """Slice/transpose the full inputs into the 8 per-core input maps."""
    import ml_dtypes
    bf = lambda a: np.ascontiguousarray(a.astype(ml_dtypes.bfloat16))
    f = lambda a: np.ascontiguousarray(np.asarray(a, dtype=np.float32))
    x, rc = f(inputs["x"]), f(inputs["rope_cache"])
    Wq, bq = f(inputs["Wq"]), f(inputs["bq"])
    Wk, bk = f(inputs["Wk"]), f(inputs["bk"])
    Wv, bv = f(inputs["Wv"]), f(inputs["bv"])
    Wo = f(inputs["Wo"])

    cos2 = np.tile(np.repeat(rc[:, 1::2].T, 2, axis=0), (2, 1))  # [128, T]
    sin2 = np.tile(np.repeat(rc[:, 0::2].T, 2, axis=0), (2, 1))
    R2 = np.zeros((128, 128), np.float32)
    for i in range(64):
        R2[2 * i, 2 * i + 1] = -1.0
        R2[2 * i + 1, 2 * i] = 1.0
    r2t = np.ascontiguousarray(R2.T)
    ident = np.eye(64, dtype=np.float32)
    kk, qq = np.arange(128)[:, None], np.arange(128)[None, :]
    dmask = np.where(kk <= qq, 0.0, NEG).astype(np.float32)

    in_maps = []
    for core in range(8):
        b, g = core // 4, core % 4
        kv = g // 2
        in_maps.append({
            "xT": bf(x[b].T.reshape(8, 128, 4, 512).transpose(1, 2, 0, 3).reshape(128, -1)),
            "wqkv": bf(np.concatenate(
                [Wv[64 * kv:64 * (kv + 1)].T, Wk[64 * kv:64 * (kv + 1)].T,
                 Wq[256 * g:256 * (g + 1), :].T],
                axis=1).reshape(8, 128, 384).transpose(1, 0, 2).reshape(128, -1)),
            "bq": np.ascontiguousarray(bq[256 * g:256 * (g + 1)].reshape(2, 128).T),
            "bkv": np.concatenate(
                [bv[64 * kv:64 * (kv + 1)], bk[64 * kv:64 * (kv + 1)]]).reshape(128, 1),
            "wo": bf(Wo[:, 256 * g:256 * (g + 1)].T.reshape(2, 128, C).transpose(1, 0, 2).reshape(128, -1)),
            "cs": bf(np.stack([cos2.reshape(128, 4, 512), sin2.reshape(128, 4, 512)],
                              axis=2).reshape(128, -1)),
            "r2t": bf(r2t),
            "ident": bf(ident), "dmask": dmask,
        })
    return in_maps


_PROGRAM = None


def _get_program():
    global _PROGRAM
    if _PROGRAM is None:
        _PROGRAM = build_program()
    return _PROGRAM


def _gather(results, bo):
    full = np.empty((B, T, C), np.float32)
    for b in range(B):
        acc = results[4 * b]["out"].astype(np.float32).copy()
        for g in range(1, 4):
            acc += results[4 * b + g]["out"]
        full[b] = acc + bo
    return full


def kernel(**inputs):
    nc = _get_program()
    in_maps = host_prep(inputs)
    res = run_bass_kernel_spmd(nc, in_maps, list(range(8)))
    return _gather(res.results, np.asarray(inputs["bo"], np.float32))


def kernel_traced(**inputs):
    """Like kernel() but with NTFF tracing; returns (output, BassKernelResults)."""
    nc = _get_program()
    in_maps = host_prep(inputs)
    res = run_bass_kernel_spmd(nc, in_maps, list(range(8)), trace=True)
    return _gather(res.results, np.asarray(inputs["bo"], np.float32)), res



# revision 13
# speedup vs baseline: 1.3193x; 1.3193x over previous
"""Causal self-attention (GQA + RoPE) on 8 Trainium2 NeuronCores.

Sharding: core c = (b, g) with b = c // 4 (batch), g = c % 4 (group of 4
consecutive Q heads; KV head g // 2). Each core computes the attention
output for its 4 heads and a partial out-projection through the matching
256-column slice of Wo. Host sums the 4 partials per batch and adds bo.

v3 design (fp8 DoubleRow heavy):
  - Q projection in fp8-DR (weights host-scaled x64), V+K projection bf16.
  - RoPE in a deinterleaved head-dim layout (host permutes Wq/Wk rows) so
    the rotation is pure DVE mul/add; outputs q8/kz8 written directly fp8.
  - Scores all fp8-DR: stationary kz8 tiles store [k-chunk | zeros] pairs
    so the DR slab-sum contracts 64 real dims at 0.5 cyc/row; the causal
    mask is accumulated into score PSUM by a tiny fp8-DR matmul
    (8*I slab x (-240) mask = -1920).
  - AV: full blocks fp8-DR (vA8/e8), diagonal band bf16 (accuracy for
    early rows). Out-projection bf16 (fp8 there breaks tolerance).
  - Emission interleaves proj(ch+1) and outproj(ch-1) into att(ch) so the
    PE stays busy (p-state) while Scalar streams the exps.
"""

import sys

for _p in ("/opt/trn_rl_repo", "/opt/pypackages"):
    if _p not in sys.path:
        sys.path.append(_p)

from contextlib import ExitStack

import numpy as np

import concourse.bacc as bacc
import concourse.mybir as mybir
import concourse.tile as tile
from concourse.bass import ts
from concourse.bass_utils import run_bass_kernel_spmd

B, T, C = 2, 2048, 1024
HQ, HKV, HD = 16, 2, 64
F32 = mybir.dt.float32
FP8 = mybir.dt.float8e4
DR = mybir.MatmulPerfMode.DoubleRow
BF16 = mybir.dt.bfloat16
AF = mybir.ActivationFunctionType
ALU = mybir.AluOpType
NCC = C // 128        # 8 chunks of the contraction dim
NC16 = T // 128       # 16 k-chunks of 128
SCALE = 1.0 / 64.0    # the reference's double 1/sqrt(64) scaling
WS = 1.0 / 64.0       # fp8 weight descale (weights host-scaled x64)


def _r2(ap):
    """[128, 256] -> [128, 2, 128] slab view for DoubleRow."""
    return ap.rearrange("p (k w) -> p k w", k=2)


def _emit(nc, tc, ctx, d):
    sing = ctx.enter_context(tc.tile_pool(name="sing", bufs=1))

    xT_sb = [sing.tile([128, NCC, 512], BF16, name=f"x{c}") for c in range(4)]
    x8_sb = [sing.tile([128, NCC, 512], FP8, name=f"x8_{c}") for c in range(4)]
    wkv_sb = sing.tile([128, NCC, 128], BF16)
    wq8_sb = sing.tile([128, NCC, 256], FP8)
    bq_sb = sing.tile([128, 2], F32)
    bkv_sb = sing.tile([128, 1], F32)
    wo_sb = sing.tile([128, 2, C], BF16)
    cs_sb = [sing.tile([128, 2, 512], BF16, name=f"cs{c}") for c in range(4)]
    id_sb = sing.tile([64, 64], BF16)
    mi8_sb = sing.tile([128, 2, 128], FP8)
    dmq_sb = sing.tile([128, 2, 128], FP8)

    kvT_sb = sing.tile([128, T], BF16)        # v at 0:64, k (pre-rope, dein) at 64:128
    qT_sb = sing.tile([128, 2, T], BF16)      # biased q pre-rope, dein pair layout
    q8_sb = sing.tile([128, 2, 4, 512], FP8)  # roped q fp8
    kz8a = sing.tile([128, NC16, 256], FP8)   # [k|0] chunks; k at parts 0:64
    kz8b = sing.tile([128, NC16, 256], FP8)   # [k|0] chunks; k at parts 64:128
    vA_sb = sing.tile([128, 16, 128], BF16)   # ones col 0, zeros, v at 64:128
    vA8_sb = sing.tile([128, 16, 128], FP8)   # fp8 copy for DoubleRow AV
    yT_sb = sing.tile([128, 2, T], BF16)      # normalized attention out

    # input DMAs, ordered so the first projection can start early
    xp = d["xT"].ap()     # [128, 4*NCC*512] (ch, cc) chunks
    x8p = d["x8"].ap()
    csp = d["cs"].ap()    # [128, 4*2*512] per-ch cos2|sinm
    for cc in range(NCC):
        nc.sync.dma_start(out=xT_sb[0][:, cc, :], in_=xp[:, ts(cc, 512)])
    nc.sync.dma_start(out=wkv_sb[:], in_=d["wkv"].ap())
    nc.sync.dma_start(out=bkv_sb[:], in_=d["bkv"].ap())
    nc.sync.dma_start(out=x8_sb[0][:], in_=x8p[:, ts(0, NCC * 512)])
    nc.sync.dma_start(out=wq8_sb[:], in_=d["wq8"].ap())
    nc.sync.dma_start(out=bq_sb[:], in_=d["bq"].ap())
    nc.sync.dma_start(out=cs_sb[0][:], in_=csp[:, ts(0, 1024)])
    nc.sync.dma_start(out=id_sb[:], in_=d["ident"].ap())
    nc.sync.dma_start(out=mi8_sb[:], in_=d["mi8"].ap())
    nc.sync.dma_start(out=dmq_sb[:], in_=d["dmq"].ap())
    for ch in range(1, 4):
        nc.sync.dma_start(out=xT_sb[ch][:], in_=xp[:, ts(ch, NCC * 512)])
        nc.sync.dma_start(out=x8_sb[ch][:], in_=x8p[:, ts(ch, NCC * 512)])
        nc.sync.dma_start(out=cs_sb[ch][:], in_=csp[:, ts(ch, 1024)])
        if ch == 1:
            nc.sync.dma_start(out=wo_sb[:], in_=d["wo"].ap())

    # preload the exp act table while the input DMAs stream
    warm = sing.tile([1, 1], F32)
    nc.scalar.activation(out=warm[:], in_=dmq_sb[0:1, 0, 0:1], func=AF.Exp, scale=1.0)

    # one-off zero/one constants (GpSimd so DVE stays free)
    nc.gpsimd.memset(kz8a[64:128, :, 0:128], 0.0)
    nc.gpsimd.memset(kz8a[:, :, 128:256], 0.0)
    nc.gpsimd.memset(kz8b[0:64, :, 0:128], 0.0)
    nc.gpsimd.memset(kz8b[:, :, 128:256], 0.0)
    nc.gpsimd.memset(vA_sb[:, :, 0:1], 1.0)
    nc.gpsimd.memset(vA_sb[:, :, 1:64], 0.0)
    nc.gpsimd.tensor_copy(vA8_sb[:, :, 0:64], vA_sb[:, :, 0:64])

    pw = ctx.enter_context(tc.tile_pool(name="pw", bufs=2, space="PSUM"))
    psp = ctx.enter_context(tc.tile_pool(name="psp", bufs=2, space="PSUM"))
    pyp = ctx.enter_context(tc.tile_pool(name="pyp", bufs=2, space="PSUM"))
    ep = ctx.enter_context(tc.tile_pool(name="ep", bufs=6))
    tmp = ctx.enter_context(tc.tile_pool(name="tmp", bufs=2))
    nrm = ctx.enter_context(tc.tile_pool(name="nrm", bufs=2))
    ost = ctx.enter_context(tc.tile_pool(name="ost", bufs=3))

    def cbc(ap):
        return ap.unsqueeze(1).broadcast_to([ap.shape[0], 2, 512])

    # ---- projection segments for chunk ch ----
    def proj_segs(ch):
        tc_cols = ts(ch, 512)
        assist = ch <= 1  # scalar engine handles evacs while its exp load is low
        segs = []

        def kv_mm():
            kvp = pw.tile([128, 512], F32, tag="w")
            for cc in range(NCC):
                nc.tensor.matmul(
                    kvp[:], wkv_sb[:, cc, :], xT_sb[ch][:, cc, :],
                    start=(cc == 0), stop=(cc == NCC - 1),
                )
            if assist:
                nc.scalar.activation(out=kvT_sb[:, tc_cols], in_=kvp[:],
                                     func=AF.Identity, bias=bkv_sb[:, 0:1])
            else:
                nc.vector.tensor_scalar_add(kvT_sb[:, tc_cols], kvp[:], bkv_sb[:, 0:1])
        segs.append(kv_mm)

        def k_rope():
            # sm holds [+sin,-sin,+sin,-sin] per 32-block so every operand is
            # read at its in0 base partition (walrus same-base-partition rule)
            cs, sm = cs_sb[ch][:, 0, :], cs_sb[ch][:, 1, :]
            t_k = tmp.tile([64, 512], BF16, tag="tk")
            u_k = tmp.tile([64, 512], BF16, tag="uk")
            nc.vector.tensor_mul(t_k[:], kvT_sb[64:128, tc_cols], cs[64:128, :])
            nc.vector.tensor_mul(u_k[0:32, :], kvT_sb[96:128, tc_cols], sm[96:128, :])
            nc.vector.tensor_mul(u_k[32:64, :], kvT_sb[64:96, tc_cols], sm[64:96, :])
            kz_out = kz8a[0:64, 4 * ch:4 * ch + 4, 0:128]
            nc.vector.tensor_add(kz_out, t_k[:].rearrange("p (a b) -> p a b", a=4),
                                 u_k[:].rearrange("p (a b) -> p a b", a=4))
            nc.sync.dma_start(out=kz8b[64:128, 4 * ch:4 * ch + 4, 0:128],
                              in_=kz8a[0:64, 4 * ch:4 * ch + 4, 0:128])
        segs.append(k_rope)

        def q_mm(j):
            qp = pw.tile([128, 512], F32, tag="w")
            for p in range(NCC // 2):
                nc.tensor.matmul(
                    qp[:], wq8_sb[:, 2 * p:2 * p + 2, ts(j, 128)],
                    x8_sb[ch][:, 2 * p:2 * p + 2, :],
                    start=(p == 0), stop=(p == NCC // 2 - 1), perf_mode=DR,
                )
            if assist:
                nc.scalar.activation(out=qT_sb[:, j, tc_cols], in_=qp[:],
                                     func=AF.Identity, bias=bq_sb[:, j:j + 1],
                                     scale=WS)
            else:
                nc.vector.tensor_scalar(qT_sb[:, j, tc_cols], qp[:], WS,
                                        bq_sb[:, j:j + 1], ALU.mult, ALU.add)
        segs.append(lambda: q_mm(0))
        segs.append(lambda: q_mm(1))

        def q_rope():
            cs, sm = cs_sb[ch][:, 0, :], cs_sb[ch][:, 1, :]
            qv = qT_sb[:, :, tc_cols]
            t_q = tmp.tile([128, 2, 512], BF16, tag="tq")
            u_q = tmp.tile([128, 2, 512], BF16, tag="uq")
            nc.vector.tensor_mul(t_q[:], qv, cbc(cs))
            # swap even<->odd 32-blocks within each head half; sm is indexed
            # at in0's partitions (sign pattern pre-arranged on host)
            for s0 in (0, 64):
                nc.vector.tensor_mul(u_q[s0:s0 + 32, :, :],
                                     qT_sb[s0 + 32:s0 + 64, :, tc_cols],
                                     cbc(sm[s0 + 32:s0 + 64, :]))
                nc.vector.tensor_mul(u_q[s0 + 32:s0 + 64, :, :],
                                     qT_sb[s0:s0 + 32, :, tc_cols],
                                     cbc(sm[s0:s0 + 32, :]))
            nc.vector.tensor_add(q8_sb[:, :, ch, :], t_q[:], u_q[:])
        segs.append(q_rope)

        def v_trans():
            for r in range(4):
                c16 = 4 * ch + r
                pv = pw.tile([128, 64], BF16, tag="w")
                nc.tensor.transpose(pv[:], kvT_sb[0:64, ts(c16, 128)], id_sb[:])
                nc.vector.tensor_copy(vA_sb[:, c16, 64:128], pv[:])
                nc.gpsimd.tensor_copy(vA8_sb[:, c16, 64:128], vA_sb[:, c16, 64:128])
        segs.append(v_trans)
        return segs

    # ---- attention for one (chunk, head) ----
    def att_head(qb, h):
        j = h // 2
        kz = kz8a if h % 2 == 0 else kz8b
        py = pyp.tile([128, 512], F32, tag="y")
        qch = q8_sb[:, j, qb, :]                  # [128, 512] roped q chunk
        # moving slab pair reads the window twice (stride-0); the second
        # slab is nulled by the zero half of the kz tiles
        qrhs = qch.unsqueeze(1).broadcast_to([128, 2, 512])
        # full 128x512 blocks below the diagonal band, in chunk pairs
        for p in range(2 * qb):
            st = psp.tile([128, 2, 512], F32, tag="s")
            for i in range(2):
                nc.tensor.matmul(
                    st[:, i, :], _r2(kz[:, 2 * p + i, :]), qrhs,
                    start=True, stop=True, perf_mode=DR,
                )
            e8 = ep.tile([128, 2, 512], FP8, tag="e8")
            nc.scalar.activation(out=e8[:], in_=st[:], func=AF.Exp, scale=SCALE)
            nc.tensor.matmul(
                py[:], vA8_sb[:, 2 * p:2 * p + 2, :], e8[:],
                start=(p == 0), stop=False, perf_mode=DR,
            )
        # diagonal band: 2 groups; group ra: windows at [0:wa], [off_b:off_b+wb]
        for ra in (0, 2):
            rb = ra + 1
            wa, wb = 512 - 128 * ra, 512 - 128 * rb
            off_b = 512 if ra == 0 else wa
            st = psp.tile([128, 1024], F32, tag="s")
            nc.tensor.matmul(
                st[:, 0:wa], _r2(kz[:, 4 * qb + ra, :]),
                qch[:, 128 * ra:512].unsqueeze(1).broadcast_to([128, 2, wa]),
                start=True, stop=True, perf_mode=DR, skip_group_check=True,
            )
            nc.tensor.matmul(
                st[:, 0:128], mi8_sb[:], dmq_sb[:],
                start=False, stop=True, perf_mode=DR, skip_group_check=True,
            )
            nc.tensor.matmul(
                st[:, off_b:off_b + wb], _r2(kz[:, 4 * qb + rb, :]),
                qch[:, 128 * rb:512].unsqueeze(1).broadcast_to([128, 2, wb]),
                start=True, stop=True, perf_mode=DR, skip_group_check=True,
            )
            nc.tensor.matmul(
                st[:, off_b:off_b + 128], mi8_sb[:], dmq_sb[:],
                start=False, stop=True, perf_mode=DR, skip_group_check=True,
            )
            we = off_b + wb
            e = ep.tile([128, 1024], BF16, tag="e")
            nc.scalar.activation(out=e[:, 0:we], in_=st[:, 0:we], func=AF.Exp,
                                 scale=SCALE)
            nc.tensor.matmul(
                py[:, 128 * ra:512], vA_sb[:, 4 * qb + ra, :], e[:, 0:wa],
                start=(qb == 0 and ra == 0), stop=False,
            )
            nc.tensor.matmul(
                py[:, 128 * rb:512], vA_sb[:, 4 * qb + rb, :], e[:, off_b:off_b + wb],
                start=False, stop=(rb == 3),
            )
        # normalize: 1/den (DVE), broadcast across partitions (GpSimd),
        # then scale while evacuating the AV PSUM (DVE)
        rdr = nrm.tile([1, 512], F32, tag="rdr")
        nc.vector.reciprocal_approx_fast(rdr[:], py[0:1, :])
        pbs = nrm.tile([64, 512], F32, tag="pbs")
        nc.gpsimd.partition_broadcast(pbs[:], rdr[:])
        b0 = (h % 2) * 64
        nc.vector.tensor_mul(
            yT_sb[b0:b0 + 64, j, ts(qb, 512)], py[64:128, :], pbs[:],
        )

    # ---- out-projection quarter (one tq of 128 t-rows) ----
    def outproj_q(tq):
        for cf in range(2):
            po = pw.tile([128, 512], F32, tag="w")
            for j in range(2):
                nc.tensor.matmul(
                    po[:], yT_sb[:, j, ts(tq, 128)], wo_sb[:, j, ts(cf, 512)],
                    start=(j == 0), stop=(j == 1),
                )
            ob = ost.tile([128, 512], BF16, tag="ob")
            nc.vector.tensor_copy(ob[:], po[:])
            nc.sync.dma_start(out=d["out"].ap()[ts(tq, 128), ts(cf, 512)], in_=ob[:])

    # ---- emission schedule ----
    for seg in proj_segs(0):
        seg()
    for ch in range(4):
        fillers = []
        if ch < 3:
            fillers.extend(proj_segs(ch + 1))
        if ch >= 1:
            for tq in range(4 * (ch - 1), 4 * ch):
                fillers.append(lambda tq=tq: outproj_q(tq))
        per_head = (len(fillers) + 3) // 4
        fi = 0
        for h in range(4):
            att_head(ch, h)
            for _ in range(per_head):
                if fi < len(fillers):
                    fillers[fi]()
                    fi += 1
        while fi < len(fillers):
            fillers[fi]()
            fi += 1
    for tq in range(12, 16):
        outproj_q(tq)


def build_program(num_devices=8):
    nc = bacc.Bacc("TRN2", target_bir_lowering=False, debug=False,
                   num_devices=num_devices)
    d = {}
    spec = [
        ("xT", [128, 4 * NCC * 512], BF16),
        ("x8", [128, 4 * NCC * 512], FP8),
        ("wkv", [128, NCC * 128], BF16),
        ("wq8", [128, NCC * 256], FP8),
        ("bq", [128, 2], F32),
        ("bkv", [128, 1], F32),
        ("wo", [128, 2 * C], BF16),
        ("cs", [128, 4 * 2 * 512], BF16),
        ("ident", [64, 64], BF16),
        ("mi8", [128, 2 * 128], FP8),
        ("dmq", [128, 2 * 128], FP8),
    ]
    for name, shape, dt in spec:
        d[name] = nc.dram_tensor(name, shape, dt, kind="ExternalInput")
    d["out"] = nc.dram_tensor("out", [T, C], BF16, kind="ExternalOutput")
    with tile.TileContext(nc) as tc, ExitStack() as ctx:
        _emit(nc, tc, ctx, d)
    nc.compile()
    return nc


def host_prep(inputs):
    """Slice/permute the full inputs into the 8 per-core input maps."""
    import ml_dtypes
    E4 = ml_dtypes.float8_e4m3
    bf = lambda a: np.ascontiguousarray(a.astype(ml_dtypes.bfloat16))
    f8c = lambda a: np.ascontiguousarray(a.astype(np.float32).astype(E4))
    f = lambda a: np.ascontiguousarray(np.asarray(a, dtype=np.float32))
    x, rc = f(inputs["x"]), f(inputs["rope_cache"])
    Wq, bq = f(inputs["Wq"]), f(inputs["bq"])
    Wk, bk = f(inputs["Wk"]), f(inputs["bk"])
    Wv, bv = f(inputs["Wv"]), f(inputs["bv"])
    Wo = f(inputs["Wo"])

    cos, sin = rc[:, 1::2], rc[:, 0::2]          # [T, 32]
    # dein partition layout: [h-even evens | h-even odds | h-odd evens | h-odd odds]
    po = np.arange(128)
    parity = po // 64                             # head within pair
    dd = 2 * (po % 32) + (po // 32) % 2           # orig hd dim
    ko = np.arange(64)
    kd = 2 * (ko % 32) + (ko // 32)               # k dein dim order (evens|odds)

    cos2 = cos.T[po % 32, :]                      # [128, T]
    # [+sin | -sin | +sin | -sin] per 32-block: each rope u-op reads this at
    # its in0 partition base, giving the right sign for the swapped half
    sinm = np.where(((po // 32) % 2 == 0)[:, None],
                    sin.T[po % 32, :], -sin.T[po % 32, :])
    cs = np.stack([cos2.reshape(128, 4, 512), sinm.reshape(128, 4, 512)],
                  axis=2).reshape(128, -1)        # [128, 4*2*512]

    ident = np.eye(64, dtype=np.float32)
    kk, qq = np.arange(128)[:, None], np.arange(128)[None, :]
    mi8 = np.zeros((128, 2, 128), np.float32)
    mi8[:, 0, :] = 8.0 * np.eye(128)
    dmq = np.zeros((128, 2, 128), np.float32)
    dmq[:, 0, :] = np.where(kk > qq, -240.0, 0.0)

    xsw = lambda a: a.T.reshape(NCC, 128, 4, 512).transpose(1, 2, 0, 3).reshape(128, -1)

    in_maps = []
    for core in range(8):
        b, g = core // 4, core % 4
        kv = g // 2
        # wkv: [Wv unperm | Wk dein] per cc chunk
        wv = Wv[64 * kv:64 * (kv + 1)].T          # [C, 64]
        wk = Wk[64 * kv:64 * (kv + 1)][kd].T      # [C, 64] dein row order
        wkv = np.concatenate([wv, wk], axis=1)    # [C, 128]
        wkv = wkv.reshape(NCC, 128, 128).transpose(1, 0, 2).reshape(128, -1)
        # wq8: x64, dein-permuted columns, [C, (cc), j*128+po]
        wq_rows = np.empty((2, 128), np.int64)
        for j in range(2):
            wq_rows[j] = 256 * g + 64 * (2 * j + parity) + dd
        wqt = (64.0 * Wq[wq_rows.reshape(-1)]).T.reshape(C, 2, 128)  # [C, j, po]
        wq8 = wqt.reshape(NCC, 128, 256).transpose(1, 0, 2).reshape(128, -1)
        bq_p = np.stack([bq[wq_rows[0]], bq[wq_rows[1]]], axis=1)    # [128, 2]
        bkv_p = np.concatenate([bv[64 * kv:64 * (kv + 1)],
                                bk[64 * kv:64 * (kv + 1)][kd]]).reshape(128, 1)
        in_maps.append({
            "xT": bf(xsw(x[b])),
            "x8": f8c(xsw(x[b])),
            "wkv": bf(wkv),
            "wq8": f8c(wq8),
            "bq": np.ascontiguousarray(bq_p),
            "bkv": np.ascontiguousarray(bkv_p),
            "wo": bf(Wo[:, 256 * g:256 * (g + 1)].T.reshape(2, 128, C)
                     .transpose(1, 0, 2).reshape(128, -1)),
            "cs": bf(cs),
            "ident": bf(ident),
            "mi8": np.ascontiguousarray(mi8.reshape(128, -1).astype(E4)),
            "dmq": np.ascontiguousarray(dmq.reshape(128, -1).astype(E4)),
        })
    return in_maps


_PROGRAM = None


def _get_program():
    global _PROGRAM
    if _PROGRAM is None:
        _PROGRAM = build_program()
    return _PROGRAM


def _gather(results, bo):
    full = np.empty((B, T, C), np.float32)
    for b in range(B):
        acc = results[4 * b]["out"].astype(np.float32).copy()
        for g in range(1, 4):
            acc += results[4 * b + g]["out"]
        full[b] = acc + bo
    return full


def kernel(**inputs):
    nc = _get_program()
    in_maps = host_prep(inputs)
    res = run_bass_kernel_spmd(nc, in_maps, list(range(8)))
    return _gather(res.results, np.asarray(inputs["bo"], np.float32))


def kernel_traced(**inputs):
    """Like kernel() but with NTFF tracing; returns (output, BassKernelResults)."""
    nc = _get_program()
    in_maps = host_prep(inputs)
    res = run_bass_kernel_spmd(nc, in_maps, list(range(8)), trace=True)
    return _gather(res.results, np.asarray(inputs["bo"], np.float32)), res
